# revision 6
# baseline (speedup 1.0000x reference)
"""Trainium2 Bass kernel for CustomISTFT (N_FFT=4096, HOP=1024, T=4096 frames).

Per core (frames sharded 512/core across 8 cores):
  Cooley-Tukey split of the 4096-point inverse DFT: k = 64*j1 + c,
  n = m1 + 64*m2.  Stage 1 contracts j1 per column c (Hermitian
  extension + twiddle mu^{m1 c} folded into host-built weights).
  Because the input spectrum is Hermitian, A[m1, 64-c] = conj(A[m1, c]),
  so only c = 0..32 is computed (33 calls instead of 64); stage 2
  contracts (re/im, c) with the conjugate fold, the window and the
  1/4096 * 4096/3 normalization all folded into w2.
  The FFT corner-turn (m1 <-> c) and the OLA layout-turn are pure
  SBUF->SBUF DMAs into pre-transposed SBUF tiles (no DRAM round trips).
  DMA triggers are split across the two HWDGE queues (sync + scalar).
  The overlap-add runs on the vector engine in [n mod 128, n div 128, t]
  layout so all shifts are in the free dimension.  The imaginary channel
  is exactly win[n]*(b0[t] + (-1)^n b2048[t])/4096 (rank-2 per parity),
  computed with K=8 matmuls that also perform its overlap-add; it is
  shipped as bf16.  z is pre-cast to bf16 on the host (the device fed
  bf16 to the matmuls anyway).  Host: shard, gather, reorder, halo-add
  between neighbor cores, exact wsum correction on the two edge blocks.
"""

import numpy as np
import ml_dtypes

N_FFT = 4096
HOP = 1024
FREQ = 2049
T_FRAMES = 4096
N_CORES = 8
T_CORE = T_FRAMES // N_CORES  # 512
L_FULL = (T_FRAMES - 1) * HOP + N_FFT
OUT_LEN = L_FULL - N_FFT

_bf16 = ml_dtypes.bfloat16


# ---------------------------------------------------------------- weights
def canonical_rows(c):
    """(ch, k) input rows consumed by the stage-1 call of column c.
    Matches the zt gather order: [res=c block (ch=0, ch=1), res=64-c block]."""
    if c == 0:
        return [(0, 64 * j1) for j1 in range(33)] + [(1, 64 * j1) for j1 in range(33)]
    if c == 32:
        return [(0, 32 + 64 * j1) for j1 in range(32)] + [
            (1, 32 + 64 * j1) for j1 in range(32)
        ]
    return (
        [(0, c + 64 * j1) for j1 in range(32)]
        + [(1, c + 64 * j1) for j1 in range(32)]
        + [(0, (64 - c) + 64 * j1) for j1 in range(32)]
        + [(1, (64 - c) + 64 * j1) for j1 in range(32)]
    )


def k_of_call(c):
    return 66 if c == 0 else (64 if c == 32 else 128)


def build_weights(window):
    """w1 [128,33,128] bf16 ([k-row, c, (ri,m1)-col], rows follow
    canonical_rows, zero-padded), w2 [66,64,64] bf16 ([(ri*33+c), m1,
    (par,nh)-col], conj-fold + window*4096/3/4096 folded),
    wim [8,1024] bf16."""
    win = window.astype(np.float64)
    mu = np.exp(2j * np.pi / 4096)
    w64c = np.exp(2j * np.pi / 64)
    m1v = np.arange(64)

    w1 = np.zeros((128, 33, 128), dtype=np.float64)
    for c in range(33):
        coef = {}
        for j1 in range(64):
            k = 64 * j1 + c
            e = w64c ** (m1v * j1)
            if k <= 2048:
                coef[(0, k)] = coef.get((0, k), 0) + e
                coef[(1, k)] = coef.get((1, k), 0) + 1j * e
            else:
                kr = 4096 - k
                coef[(0, kr)] = coef.get((0, kr), 0) + e
                coef[(1, kr)] = coef.get((1, kr), 0) - 1j * e
        tw = mu ** (m1v * c)
        rows = canonical_rows(c)
        assert set(rows) == set(coef.keys())
        for i, key in enumerate(rows):
            v = coef[key] * tw
            w1[i, c, :64] = v.real
            w1[i, c, 64:] = v.imag

    # stage 2: x[m1 + 64*m2] = (1/4096) * [ ReA[m1,0] + (-1)^m2 ReA[m1,32]
    #   + sum_{c=1..31} 2*(cos(th) ReA[m1,c] - sin(th) ImA[m1,c]) ],
    # th = 2*pi*m2*c/64, then * win[n]*4096/3.  Column cc = par*32+nh
    # with m2 = 2*nh+par.
    w2 = np.zeros((66, 64, 64), dtype=np.float64)
    cc = np.arange(64)
    m2 = 2 * (cc % 32) + (cc // 32)  # m2 per psum column
    for c in range(33):
        fac = 2.0 if 1 <= c <= 31 else 1.0
        th = 2 * np.pi * m2 * c / 64.0
        for m1 in range(64):
            n = m1 + 64 * m2
            wn = win[n] / 3.0  # win * (4096/3) / 4096
            w2[c, m1, :] = fac * np.cos(th) * wn
            w2[33 + c, m1, :] = -fac * np.sin(th) * wn

    # wim[(2r+par), i] = win[i + 1024 r]/3 * (par == i%2)
    wim = np.zeros((8, 1024), dtype=np.float64)
    iv = np.arange(1024)
    for r in range(4):
        for par in range(2):
            wim[2 * r + par] = (win[iv + 1024 * r] / 3.0) * (iv % 2 == par)
    return w1.astype(_bf16), w2.astype(_bf16), wim.astype(_bf16)


# ---------------------------------------------------------------- device program
def emit_kernel(tc, outre_ap, outim_ap, z_ap, w1_ap, w2_ap, wim_ap, T):
    """Per-core program.  T frames (multiple of 128).
    outre [128, 8, SPAD] f32:  outre[p, ih, s] =
        sum_r win*x[p + 128*ih + 1024*r, s - r]  (real channel, s in [0,T+3))
    outim [SC, 128, 1024] bf16: outim[sc, sp, i] = imag channel at block
        s = 128*sc + sp, position i."""
    import concourse.mybir as mybir
    from contextlib import ExitStack

    nc = tc.nc
    dt = mybir.dt
    f32, bf16 = dt.float32, dt.bfloat16
    SB = T + 3
    SC = (SB + 127) // 128
    SPAD = outre_ap.shape[2]
    assert SPAD >= SB and outim_ap.shape[0] == SC

    with ExitStack() as ctx:
        const = ctx.enter_context(tc.tile_pool(name="const", bufs=1))

        # ---- weights to SBUF (scalar HWDGE queue; contiguous layouts)
        w1_sb = const.tile([128, 33, 128], bf16)
        nc.scalar.dma_start(w1_sb[:], w1_ap[:])
        w2_sb = const.tile([66, 64, 64], bf16)
        nc.scalar.dma_start(w2_sb[:], w2_ap[:])
        wim_sb = const.tile([8, 1024], bf16)
        nc.scalar.dma_start(wim_sb[:], wim_ap[:])

        # persistent SBUF intermediates
        rt = const.tile([66, 64, T], bf16)  # A^T: [(ri*33+c), m1, t]
        x2 = const.tile([128, 32, T], bf16)  # [(par*64+m1), nh, t]
        sig = const.tile([128, 8, SPAD], f32)
        cve = const.tile([1, T], bf16)
        cvo = const.tile([1, T], bf16)
        b0t = const.tile([1, T], bf16)
        b2t = const.tile([1, T], bf16)
        cs = const.tile([8, SC * 128], bf16)

        zpool = ctx.enter_context(tc.tile_pool(name="zt", bufs=4))
        s1ps = ctx.enter_context(tc.tile_pool(name="s1ps", bufs=3, space="PSUM"))
        apool = ctx.enter_context(tc.tile_pool(name="aslot", bufs=4))

        nc.any.memset(sig[:], 0.0)
        nc.any.memset(cs[:], 0.0)

        # ---- stage 1: gather z rows, matmul, cast bf16, SB->SB turn into rt
        for g in range(33):
            K = k_of_call(g)
            zt = zpool.tile([128, T], bf16, tag="zt")
            if g == 0:
                nc.sync.dma_start(zt[0:66, :], z_ap[:, 0::64, :])
            elif g == 32:
                nc.sync.dma_start(zt[0:64, :], z_ap[:, 32::64, :])
            else:
                nc.sync.dma_start(zt[0:64, :], z_ap[:, g::64, :])
                nc.sync.dma_start(zt[64:128, :], z_ap[:, (64 - g) :: 64, :])
            if g == 0:
                # b0 = z[1,0,:], b2048 = z[1,2048,:] straight from DRAM
                nc.sync.dma_start(b0t[:], z_ap[1, 0:1, :])
                nc.sync.dma_start(b2t[:], z_ap[1, 2048:2049, :])
                nc.vector.tensor_add(cve[:], b0t[:], b2t[:])
                nc.vector.tensor_sub(cvo[:], b0t[:], b2t[:])
                # CS[2r+par, s] = cv_par[s - r]
                for r in range(4):
                    nc.sync.dma_start(cs[2 * r : 2 * r + 1, r : r + T], cve[:])
                    nc.sync.dma_start(cs[2 * r + 1 : 2 * r + 2, r : r + T], cvo[:])
            ps = s1ps.tile([128, T], f32, tag="s1ps")
            nc.tensor.matmul(
                ps[:], w1_sb[0:K, g, :], zt[0:K, :], start=True, stop=True
            )
            ab = apool.tile([128, T], bf16, tag="aslot")
            nc.any.tensor_copy(ab[:], ps[:])
            # SB->SB corner turn: src row (ri*64+m1) pairs with dst
            # partition {g, 33+g} free block (m1, t); descriptor = 1 t-run.
            nc.scalar.dma_start(rt[g::33, :, :], ab[:])

        # ---- stage 2: per m1 matmul over (ri, c), SB->SB turn into x2;
        #      imag-channel blocks interleaved to fill tensor gaps
        s2ps = ctx.enter_context(tc.tile_pool(name="s2ps", bufs=3, space="PSUM"))
        xpool = ctx.enter_context(tc.tile_pool(name="xslot", bufs=4))
        impool = ctx.enter_context(tc.tile_pool(name="imps", bufs=2, space="PSUM"))
        imsb = ctx.enter_context(tc.tile_pool(name="imsb", bufs=2))

        def emit_im_block(sc):
            it = imsb.tile([128, 1024], bf16, tag="imsb")
            for half in range(2):
                ips = impool.tile([128, 512], f32, tag="imps")
                nc.tensor.matmul(
                    ips[:],
                    cs[:, sc * 128 : (sc + 1) * 128],
                    wim_sb[:, 512 * half : 512 * (half + 1)],
                    start=True,
                    stop=True,
                )
                nc.any.tensor_copy(it[:, 512 * half : 512 * (half + 1)], ips[:])
            nc.sync.dma_start(outim_ap[sc], it[:])

        im_next = 0
        for m1 in range(64):
            ps2 = s2ps.tile([64, T], f32, tag="s2ps")
            nc.tensor.matmul(
                ps2[:], w2_sb[:, m1, :], rt[:, m1, :], start=True, stop=True
            )
            xs = xpool.tile([64, T], bf16, tag="xslot")
            nc.any.tensor_copy(xs[:], ps2[:])
            # SB->SB layout turn: src row (par*32+nh) -> dst partition
            # {m1, 64+m1} free block (nh, t).
            nc.sync.dma_start(x2[m1::64, :, :], xs[:])
            if m1 % 12 == 11 and im_next < SC:
                emit_im_block(im_next)
                im_next += 1
        while im_next < SC:
            emit_im_block(im_next)
            im_next += 1

        # ---- OLA (real): sig[p, ih, s] += x2[p, ih + 8r, s - r]
        for r in range(4):
            nc.vector.tensor_add(
                sig[:, :, r : r + T],
                sig[:, :, r : r + T],
                x2[:, 8 * r : 8 * r + 8, :],
            )
        nc.sync.dma_start(outre_ap[:], sig[:])


# ---------------------------------------------------------------- build + run
_CACHE = {}
SPAD = 520  # padded s extent of outre (>= T_CORE + 3)


def _build(T):
    import concourse.bacc as bacc
    import concourse.tile as tile
    import concourse.mybir as mybir

    dt = mybir.dt
    SC = (T + 3 + 127) // 128
    nc = bacc.Bacc("TRN2", target_bir_lowering=False, debug=False, num_devices=N_CORES)
    z_t = nc.dram_tensor("z", [2, FREQ, T], dt.bfloat16, kind="ExternalInput")
    w1_t = nc.dram_tensor("w1", [128, 33, 128], dt.bfloat16, kind="ExternalInput")
    w2_t = nc.dram_tensor("w2", [66, 64, 64], dt.bfloat16, kind="ExternalInput")
    wim_t = nc.dram_tensor("wim", [8, 1024], dt.bfloat16, kind="ExternalInput")
    spad = max(SPAD, T + 3)
    outre_t = nc.dram_tensor("outre", [128, 8, spad], dt.float32, kind="ExternalOutput")
    outim_t = nc.dram_tensor(
        "outim", [SC, 128, 1024], dt.bfloat16, kind="ExternalOutput"
    )
    with tile.TileContext(nc) as tc:
        emit_kernel(
            tc, outre_t.ap(), outim_t.ap(), z_t.ap(), w1_t.ap(), w2_t.ap(),
            wim_t.ap(), T,
        )
    nc.compile()
    return nc


def core_out_to_sig(outre, outim, T):
    """[128,8,spad] f32 + [SC,128,1024] bf16 -> [2, (T+3)*1024] f32."""
    SB = T + 3
    re = outre.transpose(2, 1, 0).reshape(-1, 1024)[:SB]  # [s, i]
    im = np.asarray(outim, dtype=np.float32).reshape(-1, 1024)[:SB]
    return np.stack([re.reshape(-1), im.reshape(-1)])


def make_in_maps(z, window):
    """Shard full f32 inputs into per-core bf16 in_maps."""
    zb = np.asarray(z, dtype=np.float32).astype(_bf16)
    wkey = window.tobytes()
    if _CACHE.get("wkey") != wkey:
        _CACHE["weights"] = build_weights(np.asarray(window, dtype=np.float32))
        _CACHE["wkey"] = wkey
    w1, w2, wim = _CACHE["weights"]
    in_maps = []
    for m in range(N_CORES):
        zc = np.ascontiguousarray(zb[:, :, m * T_CORE : (m + 1) * T_CORE])
        in_maps.append({"z": zc, "w1": w1, "w2": w2, "wim": wim})
    return in_maps


def kernel(z, window):
    from concourse.bass_utils import run_bass_kernel_spmd

    z = np.asarray(z, dtype=np.float32)
    window = np.asarray(window, dtype=np.float32)
    assert z.shape == (2, FREQ, T_FRAMES)

    if "nc" not in _CACHE:
        _CACHE["nc"] = _build(T_CORE)
    nc = _CACHE["nc"]

    in_maps = make_in_maps(z, window)
    res = run_bass_kernel_spmd(nc, in_maps, core_ids=list(range(N_CORES)))

    full = np.zeros((2, L_FULL), dtype=np.float32)
    span = (T_CORE + 3) * 1024
    for m in range(N_CORES):
        o = core_out_to_sig(res.results[m]["outre"], res.results[m]["outim"], T_CORE)
        full[:, m * T_CORE * HOP : m * T_CORE * HOP + span] += o
    out = full[:, N_FFT // 2 : L_FULL - N_FFT // 2]

    win = window.astype(np.float64)
    ws_start = win[0:1024] + win[1024:2048] + win[2048:3072]
    ws_end = win[1024:2048] + win[2048:3072] + win[3072:4096]
    out[:, :1024] *= ((3.0 / 4096.0) / ws_start).astype(np.float32)[None, :]
    out[:, -1024:] *= ((3.0 / 4096.0) / ws_end).astype(np.float32)[None, :]
    return out
